# revision 26
# baseline (speedup 1.0000x reference)
"""Trainium2 Bass kernel for AtlasMemoryPoly (dense_mlp).

Reference (DIM=256, HIDDEN=1024, POLY=33152, x:(2,1024,256)):
    x_poly = [x, x_i*x_j for i<=j]                  # (T=2048, P=33152)
    gate   = silu(x_poly @ w2.T)                    # (T, H)
    value  = x_poly @ w3.T                          # (T, H)
    out    = x + (gate*value) @ w1.T                # (T, D)

Sharding: 8 cores = 4 t-groups (512 each) x 2 h-groups (512 each).
Each core computes its (t_local, h_local) block and a partial output
(512, 256); the host sums the 2 h-group partials per t-group, adds x.

Poly features: the 33152 poly axis is PERMUTED host-side (same
permutation applied to w2/w3 rows) into PAIRS of 128-feature tiles.
Pair q covers tiles (2q, 2q+1), generated as elementwise multiplies
of stacked row-window tiles (partition p, j in {0,1}):
    q0: [X0;X1] copy              (linear features)
    q1: [X0;X1] * [X0;X1]         (squares)
    q2: [X0*X1 ; ZERO-PAD]        (antipodal + pad to 260 tiles)
    q(2+d), d=1..127: [X0;X1] * xt2[d:256+d]
          j=0: X0*xT[d:d+128]      -> pairs (i, i+d)
          j=1: X1*xT[128+d:256+d]  -> pairs (128+i, (128+i+d)%256)
xt2 is xT doubled (512 rows) so every window is a strided DMA gather.

Processing order (worder): [q0,q1,q2,q66] then superbatches (d, d+64)
for d=1..63 — each superbatch is ONE 4-D strided DMA (overlapping
windows d and d+64 expressed via a hand-built access pattern), ONE DVE
multiply against XX2=[XX|XX], and ONE contiguous 2-pair weight DMA per
w (host packs w2/w3 pair-blocks in worder). This halves DMA instruction
counts and doubles HWDGE in-flight slack vs per-pair transfers.

All products are written fp8e4 and consumed by DoubleRow matmuls
(K=256, 2 MACs/PE/cycle — the fp8 peak). w2/w3 are host-scaled by 256
(fp8e4 range); the gate path descales via ACT silu scale, the value
path's 1/256 is folded into w1.

Head: XX is loaded as 4 chunks, one leading each DMA queue, so the
weight-prefetch flood can't starve it; XX2 is built SBUF->SBUF. A few
dummy warm-up matmuls run during the DMA head to engage the PE HAM
clock-gate early. Tail: hc-outer output matmuls overlap the epilogue,
PSUM drains alternate ACT/DVE, output DMAs avoid the gpsimd queue so
its slow end-of-kernel drain hides under the tail.
"""

import sys

sys.path.insert(0, "/opt/trn_rl_repo")

import numpy as np
import ml_dtypes

DIM = 256
HIDDEN = 1024
T = 2048
POLY = DIM + DIM * (DIM + 1) // 2  # 33152
NPAIR = 130  # 260 tiles of 128 (one zero pad tile)
NCORES = 8
NHG = 2
NTG = 4
HLOC = HIDDEN // NHG  # 512
TLOC = T // NTG  # 512
NHC = HLOC // 128  # 4 h-chunks
W_SCALE = 256.0
WARMUP_MM = 7  # dummy matmuls during the DMA head to pre-warm the PE clock gate
# Ring depths bound the HEAD's HBM flood (every free slot issues its DMA at
# t=0, competing with the critical XX/shift chain) while steady-state lead
# stays ample: one kpair is consumed per ~3.4us, so ring 4 = ~13.8us of lead
# vs ~1.3us transfer time.
WTS_BUFS = 4  # weight kpair-tile prefetch ring depth (2KB/partition each)
SHIFT_BUFS = 5  # shift superbatch-window ring depth
POLY_BUFS = 8  # poly product-tile ring depth

# pair processing order: specials, then superbatch pairs (d, d+64), d=1..63
WORDER = [0, 1, 2, 66] + [q for d in range(1, 64) for q in (d + 2, d + 66)]

BF16 = ml_dtypes.bfloat16
FP8 = ml_dtypes.float8_e4m3fn


def build_perm():
    """tile-row index (260*128) -> old poly row, or -1 for the pad tile.

    Tile order: [lin0, lin1, sq0, sq1, anti, PAD, then (A_d, B_d) for
    d=1..127] where A_d rows i are pairs (i, i+d) and B_d rows i are
    pairs (128+i, (128+i+d) % 256).
    """
    i = np.arange(128)

    def pairs_to_old(a, b):
        lo = np.minimum(a, b)
        hi = np.maximum(a, b)
        return DIM + lo * DIM - lo * (lo - 1) // 2 + (hi - lo)

    chunks = [
        np.arange(0, 128),                  # lin0
        np.arange(128, 256),                # lin1
        pairs_to_old(i, i),                 # sq0
        pairs_to_old(128 + i, 128 + i),     # sq1
        pairs_to_old(i, 128 + i),           # anti
        np.full(128, -1, dtype=np.int64),   # PAD
    ]
    for d in range(1, 128):
        chunks.append(pairs_to_old(i, i + d))
        j = (128 + i + d) % 256
        chunks.append(pairs_to_old(128 + i, j))
    return np.concatenate(chunks)


_NC_CACHE = None


def _build_nc():
    from concourse import bacc, tile, mybir
    from concourse.mybir import ActivationFunctionType as AF
    from bass_rust import VecI64Pair

    nc = bacc.Bacc()
    bf = mybir.dt.bfloat16
    f8 = mybir.dt.float8e4
    f32 = mybir.dt.float32
    DR = mybir.MatmulPerfMode.DoubleRow

    xt_d = nc.dram_tensor("xt", (2 * DIM, TLOC), f8, kind="ExternalInput")
    wf8_d = nc.dram_tensor("wf8", (2, NPAIR, 128, 2, HLOC), f8, kind="ExternalInput")
    w1_d = nc.dram_tensor("w1s", (HLOC, DIM), bf, kind="ExternalInput")
    out_d = nc.dram_tensor("out", (TLOC, DIM), f32, kind="ExternalOutput")

    def shift_src(d):
        """3-D DRAM gather: rows d + p + 64*m (m=0..3), cols t.

        With tile layout (p, j, s, t) — j outer, s inner — m = 2j+s, so
        windows d (s=0) and d+64 (s=1) interleave affinely: m=0 -> window
        d half j0, m=1 -> window d+64 half j0, m=2 -> d half j1, m=3 ->
        d+64 half j1. Overlapping windows in one DMA via a hand-built AP.
        """
        ap = xt_d[d : d + 128, :].unsqueeze(1).copy()
        ap.ap = VecI64Pair([[TLOC, 128], [64 * TLOC, 4], [1, TLOC]])
        return ap

    with tile.TileContext(nc) as tc:
        with (
            tc.tile_pool(name="xpool", bufs=1) as xpool,
            tc.tile_pool(name="shift", bufs=SHIFT_BUFS) as shift,
            tc.tile_pool(name="poly", bufs=POLY_BUFS) as poly,
            tc.tile_pool(name="wts", bufs=WTS_BUFS) as wts,
            tc.tile_pool(name="epi", bufs=1) as epi,
            tc.tile_pool(name="ostage", bufs=4) as ostage,
            tc.tile_pool(name="psum", bufs=1, space="PSUM") as psum,
        ):
            # XX = [X0; X1] stacked pair tile (128, 2*TLOC): j-major halves.
            # One DMA leading the sync queue (everything depends on it; the
            # weight queues' floods are bounded by their ring depths).
            XX = xpool.tile([128, 2 * TLOC], f8, tag="XX")
            XXv = XX.rearrange("p (j t) -> p j t", j=2)
            nc.sync.dma_start(
                XXv[:], xt_d[0:256, :].rearrange("(j p) t -> p j t", p=128)
            )
            # XX2: j-outer/s-inner interleave [X0|X0|X1|X1] matching the
            # superbatch window layout. Built by the DVE (keeps both the HBM
            # and the DMA queues out of the critical head path); emitted
            # after pt1 below so q1's product comes first.
            XX2 = xpool.tile([128, 4 * TLOC], f8, tag="XX2")
            XX2v = XX2.rearrange("p (j s t) -> p j s t", j=2, s=2)

            acc = {}
            for w in (0, 1):
                for hc in range(NHC):
                    acc[(w, hc)] = psum.tile(
                        [128, TLOC], f32, tag=f"acc{w}{hc}", name=f"acc{w}{hc}"
                    )

            if WARMUP_MM:
                # PE HAM clock-gate warmup: dummy matmuls on a zeroed tile
                # while the first DMAs are in flight. q0's start=True
                # re-clears the accumulator, so results are discarded.
                wu = xpool.tile([128, 512], bf, tag="warmup")
                nc.gpsimd.memset(wu[:], 0.0)
                for i in range(WARMUP_MM):
                    nc.tensor.matmul(
                        acc[(0, 0)][:],
                        wu[:, 0:128],
                        wu[:],
                        start=True,
                        stop=True,
                        skip_group_check=True,
                    )

            # w1 tiles are loaded mid-stream (see kpair loop) so their DMAs
            # don't compete with XX/first-weight tiles for head HBM bandwidth.
            w1t = {}

            def load_w1():
                for hc in range(NHC):
                    wt1 = xpool.tile([128, DIM], bf, tag=f"w1_{hc}", name=f"w1_{hc}")
                    nc.gpsimd.dma_start(wt1[:], w1_d[hc * 128 : (hc + 1) * 128, :])
                    w1t[hc] = wt1

            def load_wpair(k):
                """One contiguous 2-pair weight DMA per w (scalar / gpsimd)."""
                wtiles = []
                for w in (0, 1):
                    wt = wts.tile(
                        [128, 2, 2, HLOC], f8, tag=f"wf8_{w}", name=f"wf{w}_{k}"
                    )
                    eng = nc.gpsimd if w == 1 else nc.scalar
                    eng.dma_start(
                        wt[:],
                        wf8_d[w, 2 * k : 2 * k + 2].rearrange("k p j h -> p k j h"),
                    )
                    wtiles.append(wt)
                return wtiles

            def consume(q, pt3, wtiles, idx, ws=(0, 1)):
                """DR matmuls accumulating pair q from wtiles[w][:, idx]."""
                st = q == 0
                sp = q == NPAIR - 1
                # hc-major on the last pair so the per-hc epilogue chains can
                # start as soon as their accumulators stop.
                order = (
                    [(w, hc) for hc in range(NHC) for w in ws]
                    if sp
                    else [(w, hc) for w in ws for hc in range(NHC)]
                )
                for w, hc in order:
                    hsl = slice(hc * 128, (hc + 1) * 128)
                    nc.tensor.matmul(
                        acc[(w, hc)][:],
                        wtiles[w][:, idx, :, hsl],
                        pt3[:],
                        start=st,
                        stop=sp,
                        perf_mode=DR,
                    )

            # kpairs 0+1 products: q1 squares, then XX2 (DVE-built), q2, q66.
            # Their matmuls interleave w2-before-w3 across both kpairs so the
            # PE has ~3.4us of w2 work while the first gpsimd (w3) weight
            # transfers land.
            wts_k0 = load_wpair(0)
            wts_k1 = load_wpair(1)
            pt1 = poly.tile([128, 2 * TLOC], f8, tag="poly", name="pt1")
            nc.vector.tensor_mul(pt1[:], XX[:], XX[:])
            # XX2 halves on the gpsimd ENGINE, in parallel with the DVE's
            # pt1/pt2/pt66 chain, so ps1 isn't serialized behind them.
            for s in (0, 1):
                nc.gpsimd.tensor_copy(XX2v[:, :, s], XXv[:])
            pt2 = poly.tile([128, 2 * TLOC], f8, tag="poly", name="pt2")
            nc.vector.tensor_mul(pt2[:, 0:TLOC], XX[:, 0:TLOC], XX[:, TLOC : 2 * TLOC])
            nc.gpsimd.memset(pt2[:, TLOC : 2 * TLOC], 0.0)
            sw64 = shift.tile([128, 2 * TLOC], f8, tag="sd", name="sw64")
            nc.sync.dma_start(
                sw64.rearrange("p (j t) -> p j t", j=2),
                xt_d[64 : 64 + 256, :].rearrange("(j p) t -> p j t", p=128),
            )
            pt66 = poly.tile([128, 2 * TLOC], f8, tag="poly", name="pt66")
            nc.vector.tensor_mul(pt66[:], XX[:], sw64[:])
            pt1v = pt1.rearrange("p (j t) -> p j t", j=2)
            pt2v = pt2.rearrange("p (j t) -> p j t", j=2)
            pt66v = pt66.rearrange("p (j t) -> p j t", j=2)
            consume(0, XXv, wts_k0, 0, ws=(0,))
            consume(1, pt1v, wts_k0, 1, ws=(0,))
            consume(0, XXv, wts_k0, 0, ws=(1,))
            consume(1, pt1v, wts_k0, 1, ws=(1,))
            consume(2, pt2v, wts_k1, 0, ws=(0,))
            consume(66, pt66v, wts_k1, 1, ws=(0,))
            consume(2, pt2v, wts_k1, 0, ws=(1,))
            consume(66, pt66v, wts_k1, 1, ws=(1,))

            # superbatches (d, d+64), d = 1..63
            for d in range(1, 64):
                k = d + 1
                if k == 12:
                    load_w1()
                wts_k = load_wpair(k)
                sw = shift.tile([128, 4 * TLOC], f8, tag="sd", name=f"sw{d}")
                nc.sync.dma_start(
                    sw.rearrange("p (m t) -> p m t", m=4), shift_src(d)
                )
                ps = poly.tile([128, 4 * TLOC], f8, tag="poly", name=f"ps{d}")
                psv = ps.rearrange("p (j s t) -> p j s t", j=2, s=2)
                if d == 1:
                    # first superbatch: two s-half multiplies straight off XX,
                    # so ps1 doesn't wait for the XX2 build
                    swv = sw.rearrange("p (j s t) -> p j s t", j=2, s=2)
                    for s in (0, 1):
                        nc.vector.tensor_mul(psv[:, :, s], XXv[:], swv[:, :, s])
                else:
                    nc.vector.tensor_mul(ps[:], XX2[:], sw[:])
                consume(d + 2, psv[:, :, 0], wts_k, 0)
                consume(d + 66, psv[:, :, 1], wts_k, 1)

            # epilogue per h-chunk: gated = silu(gate/256) * value_raw, bf16.
            # value's 1/256 is folded into w1 host-side, so the DVE multiply
            # reads acc1 (PSUM) directly — no tensor_scalar pass.
            gated = {}
            for hc in range(NHC):
                sil = epi.tile([128, TLOC], bf, tag=f"sil{hc}", name=f"sil{hc}")
                g = epi.tile([128, TLOC], bf, tag=f"gated{hc}", name=f"g{hc}")
                nc.scalar.activation(
                    sil[:], acc[(0, hc)][:], AF.Silu, scale=1.0 / W_SCALE
                )
                nc.vector.tensor_mul(g[:], sil[:], acc[(1, hc)][:])
                gated[hc] = g

            # out matmuls hc-outer: wave hc starts as soon as gated[hc] exists
            NTC = TLOC // 128
            ops = {}
            for tc_i in range(NTC):
                ops[tc_i] = psum.tile(
                    [128, DIM],
                    f32,
                    tag=f"acc{tc_i % 2}{(tc_i // 2) % 2}",
                    name=f"ops{tc_i}",
                )
            for hc in range(NHC):
                for tc_i in range(NTC):
                    tsl = slice(tc_i * 128, (tc_i + 1) * 128)
                    nc.tensor.matmul(
                        ops[tc_i][:],
                        gated[hc][:, tsl],
                        w1t[hc][:],
                        start=hc == 0,
                        stop=hc == NHC - 1,
                    )
            # stage + store: alternate ACT/DVE for the PSUM reads. Output DMAs
            # avoid the gpsimd queue so its slow end-of-kernel drain starts
            # right after the last w3 tile and hides under the tail.
            oq = [nc.sync, nc.scalar, nc.sync, nc.scalar]
            for tc_i in range(NTC):
                tsl = slice(tc_i * 128, (tc_i + 1) * 128)
                ost = ostage.tile([128, DIM], f32, tag="ost", name=f"ost{tc_i}")
                if tc_i % 2 == 0:
                    nc.scalar.copy(ost[:], ops[tc_i][:])
                else:
                    nc.vector.tensor_copy(ost[:], ops[tc_i][:])
                oq[tc_i % 4].dma_start(out_d[tsl, :], ost[:])

    nc.finalize()
    return nc


def _get_nc():
    global _NC_CACHE
    if _NC_CACHE is None:
        _NC_CACHE = _build_nc()
    return _NC_CACHE


def prepare_inputs(x, w1, w2, w3):
    """Host-side shard prep. Returns in_maps for the 8 cores."""
    perm = build_perm()  # (260*128,) with -1 for pad rows
    xt1 = np.ascontiguousarray(x.reshape(T, DIM).T).astype(FP8)  # (256, 2048)
    xt2 = np.concatenate([xt1, xt1], axis=0)  # (512, 2048)
    worder = np.asarray(WORDER)

    def to_pairs(w):  # (HIDDEN, POLY) -> (NPAIR, 128, 2, HIDDEN) f32 scaled
        wt = w.T * W_SCALE  # (POLY, HIDDEN)
        wt = np.concatenate([wt, np.zeros((1, HIDDEN), wt.dtype)], axis=0)
        g = wt[perm]  # perm -1 -> last (zero) row
        # row layout: pair q, tile j, partition k  ->  row (2q+j)*128+k
        pairs = g.reshape(NPAIR, 2, 128, HIDDEN).transpose(0, 2, 1, 3)
        return pairs[worder]  # device processing order

    w2p = to_pairs(w2)
    w3p = to_pairs(w3)
    # value path's 1/W_SCALE descale is folded into w1 (the epilogue multiplies
    # silu(gate) by the raw, W_SCALE-scaled value accumulator)
    w1t = np.ascontiguousarray(w1.T / W_SCALE).astype(BF16)  # (1024, 256)

    in_maps = []
    for c in range(NCORES):
        tg, hg = divmod(c, NHG)
        tsl = slice(tg * TLOC, (tg + 1) * TLOC)
        hsl = slice(hg * HLOC, (hg + 1) * HLOC)
        wf8 = np.stack([w2p[:, :, :, hsl], w3p[:, :, :, hsl]]).astype(FP8)
        in_maps.append(
            {
                "xt": np.ascontiguousarray(xt2[:, tsl]),
                "wf8": np.ascontiguousarray(wf8),
                "w1s": np.ascontiguousarray(w1t[hsl, :]),
            }
        )
    return in_maps


def run(x, w1, w2, w3, trace=False, trace_kwargs=None):
    from concourse.bass_utils import run_bass_kernel_spmd

    nc = _get_nc()
    in_maps = prepare_inputs(x, w1, w2, w3)
    last_err = None
    for attempt in range(3):
        try:
            res = run_bass_kernel_spmd(
                nc,
                in_maps,
                core_ids=list(range(NCORES)),
                trace=trace,
                **(trace_kwargs or {}),
            )
            break
        except Exception as e:  # transient device wedge (e.g. NRT unrecoverable)
            last_err = e
            import time as _time

            _time.sleep(5)
    else:
        raise last_err
    out = np.empty((T, DIM), dtype=np.float64)
    for tg in range(NTG):
        tsl = slice(tg * TLOC, (tg + 1) * TLOC)
        accs = np.zeros((TLOC, DIM), dtype=np.float64)
        for hg in range(NHG):
            accs += res.results[tg * NHG + hg]["out"].astype(np.float64)
        out[tsl] = x.reshape(T, DIM)[tsl].astype(np.float64) + accs
    return out.astype(np.float32).reshape(x.shape), res


def kernel(x, w1, w2, w3):
    out, _ = run(np.asarray(x), np.asarray(w1), np.asarray(w2), np.asarray(w3))
    return out


# revision 28
# speedup vs baseline: 1.0152x; 1.0152x over previous
"""Trainium2 Bass kernel for AtlasMemoryPoly (dense_mlp).

Reference (DIM=256, HIDDEN=1024, POLY=33152, x:(2,1024,256)):
    x_poly = [x, x_i*x_j for i<=j]                  # (T=2048, P=33152)
    gate   = silu(x_poly @ w2.T)                    # (T, H)
    value  = x_poly @ w3.T                          # (T, H)
    out    = x + (gate*value) @ w1.T                # (T, D)

Sharding: 8 cores = 4 t-groups (512 each) x 2 h-groups (512 each).
Each core computes its (t_local, h_local) block and a partial output
(512, 256); the host sums the 2 h-group partials per t-group, adds x.

Poly features: the 33152 poly axis is PERMUTED host-side (same
permutation applied to w2/w3 rows) into PAIRS of 128-feature tiles.
Pair q covers tiles (2q, 2q+1), generated as elementwise multiplies
of stacked row-window tiles (partition p, j in {0,1}):
    q0: [X0;X1] copy              (linear features)
    q1: [X0;X1] * [X0;X1]         (squares)
    q2: [X0*X1 ; ZERO-PAD]        (antipodal + pad to 260 tiles)
    q(2+d), d=1..127: [X0;X1] * xt2[d:256+d]
          j=0: X0*xT[d:d+128]      -> pairs (i, i+d)
          j=1: X1*xT[128+d:256+d]  -> pairs (128+i, (128+i+d)%256)
xt2 is xT doubled (512 rows) so every window is a strided DMA gather.

Processing order (worder): [q0,q1,q2,q66] then superbatches (d, d+64)
for d=1..63 — each superbatch is ONE 4-D strided DMA (overlapping
windows d and d+64 expressed via a hand-built access pattern), ONE DVE
multiply against XX2=[XX|XX], and ONE contiguous 2-pair weight DMA per
w (host packs w2/w3 pair-blocks in worder). This halves DMA instruction
counts and doubles HWDGE in-flight slack vs per-pair transfers.

All products are written fp8e4 and consumed by DoubleRow matmuls
(K=256, 2 MACs/PE/cycle — the fp8 peak). w2/w3 are host-scaled by 256
(fp8e4 range); the gate path descales via ACT silu scale, the value
path's 1/256 is folded into w1.

Head: XX is loaded as 4 chunks, one leading each DMA queue, so the
weight-prefetch flood can't starve it; XX2 is built SBUF->SBUF. A few
dummy warm-up matmuls run during the DMA head to engage the PE HAM
clock-gate early. Tail: hc-outer output matmuls overlap the epilogue,
PSUM drains alternate ACT/DVE, output DMAs avoid the gpsimd queue so
its slow end-of-kernel drain hides under the tail.
"""

import sys

sys.path.insert(0, "/opt/trn_rl_repo")

import numpy as np
import ml_dtypes

DIM = 256
HIDDEN = 1024
T = 2048
POLY = DIM + DIM * (DIM + 1) // 2  # 33152
NPAIR = 130  # 260 tiles of 128 (one zero pad tile)
NCORES = 8
NHG = 2
NTG = 4
HLOC = HIDDEN // NHG  # 512
TLOC = T // NTG  # 512
NHC = HLOC // 128  # 4 h-chunks
W_SCALE = 256.0
WARMUP_MM = 7  # dummy matmuls during the DMA head to pre-warm the PE clock gate
# Ring depths bound the HEAD's HBM flood (every free slot issues its DMA at
# t=0, competing with the critical XX/shift chain) while steady-state lead
# stays ample: one kpair is consumed per ~3.4us, so ring 4 = ~13.8us of lead
# vs ~1.3us transfer time.
WTS_BUFS = 4  # weight kpair-tile prefetch ring depth (2KB/partition each)
SHIFT_BUFS = 5  # shift superbatch-window ring depth
POLY_BUFS = 8  # poly product-tile ring depth

# pair processing order: specials, then superbatch pairs (d, d+64), d=1..63
WORDER = [0, 1, 2, 66] + [q for d in range(1, 64) for q in (d + 2, d + 66)]

BF16 = ml_dtypes.bfloat16
FP8 = ml_dtypes.float8_e4m3fn


def build_perm():
    """tile-row index (260*128) -> old poly row, or -1 for the pad tile.

    Tile order: [lin0, lin1, sq0, sq1, anti, PAD, then (A_d, B_d) for
    d=1..127] where A_d rows i are pairs (i, i+d) and B_d rows i are
    pairs (128+i, (128+i+d) % 256).
    """
    i = np.arange(128)

    def pairs_to_old(a, b):
        lo = np.minimum(a, b)
        hi = np.maximum(a, b)
        return DIM + lo * DIM - lo * (lo - 1) // 2 + (hi - lo)

    chunks = [
        np.arange(0, 128),                  # lin0
        np.arange(128, 256),                # lin1
        pairs_to_old(i, i),                 # sq0
        pairs_to_old(128 + i, 128 + i),     # sq1
        pairs_to_old(i, 128 + i),           # anti
        np.full(128, -1, dtype=np.int64),   # PAD
    ]
    for d in range(1, 128):
        chunks.append(pairs_to_old(i, i + d))
        j = (128 + i + d) % 256
        chunks.append(pairs_to_old(128 + i, j))
    return np.concatenate(chunks)


_NC_CACHE = None


def _build_nc():
    from concourse import bacc, tile, mybir
    from concourse.mybir import ActivationFunctionType as AF
    from bass_rust import VecI64Pair

    nc = bacc.Bacc()
    bf = mybir.dt.bfloat16
    f8 = mybir.dt.float8e4
    f32 = mybir.dt.float32
    DR = mybir.MatmulPerfMode.DoubleRow

    xt_d = nc.dram_tensor("xt", (2 * DIM, TLOC), f8, kind="ExternalInput")
    wf8_d = nc.dram_tensor("wf8", (2, NPAIR, 128, 2, HLOC), f8, kind="ExternalInput")
    w1_d = nc.dram_tensor("w1s", (HLOC, DIM), bf, kind="ExternalInput")
    out_d = nc.dram_tensor("out", (TLOC, DIM), f32, kind="ExternalOutput")

    def shift_src(d):
        """3-D DRAM gather: rows d + p + 64*m (m=0..3), cols t.

        With tile layout (p, j, s, t) — j outer, s inner — m = 2j+s, so
        windows d (s=0) and d+64 (s=1) interleave affinely: m=0 -> window
        d half j0, m=1 -> window d+64 half j0, m=2 -> d half j1, m=3 ->
        d+64 half j1. Overlapping windows in one DMA via a hand-built AP.
        """
        ap = xt_d[d : d + 128, :].unsqueeze(1).copy()
        ap.ap = VecI64Pair([[TLOC, 128], [64 * TLOC, 4], [1, TLOC]])
        return ap

    with tile.TileContext(nc) as tc:
        with (
            tc.tile_pool(name="xpool", bufs=1) as xpool,
            tc.tile_pool(name="shift", bufs=SHIFT_BUFS) as shift,
            tc.tile_pool(name="poly", bufs=POLY_BUFS) as poly,
            tc.tile_pool(name="wts", bufs=WTS_BUFS) as wts,
            tc.tile_pool(name="epi", bufs=1) as epi,
            tc.tile_pool(name="ostage", bufs=4) as ostage,
            tc.tile_pool(name="psum", bufs=1, space="PSUM") as psum,
        ):
            # XX = [X0; X1] stacked pair tile (128, 2*TLOC): j-major halves.
            # One DMA leading the sync queue (everything depends on it; the
            # weight queues' floods are bounded by their ring depths).
            XX = xpool.tile([128, 2 * TLOC], f8, tag="XX")
            XXv = XX.rearrange("p (j t) -> p j t", j=2)
            nc.sync.dma_start(
                XXv[:], xt_d[0:256, :].rearrange("(j p) t -> p j t", p=128)
            )
            # XX2: j-outer/s-inner interleave [X0|X0|X1|X1] matching the
            # superbatch window layout. Built by the DVE (keeps both the HBM
            # and the DMA queues out of the critical head path); emitted
            # after pt1 below so q1's product comes first.
            XX2 = xpool.tile([128, 4 * TLOC], f8, tag="XX2")
            XX2v = XX2.rearrange("p (j s t) -> p j s t", j=2, s=2)

            acc = {}
            for w in (0, 1):
                for hc in range(NHC):
                    acc[(w, hc)] = psum.tile(
                        [128, TLOC], f32, tag=f"acc{w}{hc}", name=f"acc{w}{hc}"
                    )

            if WARMUP_MM:
                # PE HAM clock-gate warmup: dummy matmuls on a zeroed tile
                # while the first DMAs are in flight. q0's start=True
                # re-clears the accumulator, so results are discarded.
                wu = xpool.tile([128, 512], bf, tag="warmup")
                nc.gpsimd.memset(wu[:], 0.0)
                for i in range(WARMUP_MM):
                    nc.tensor.matmul(
                        acc[(0, 0)][:],
                        wu[:, 0:128],
                        wu[:],
                        start=True,
                        stop=True,
                        skip_group_check=True,
                    )

            # w1 tiles are loaded mid-stream (see kpair loop) so their DMAs
            # don't compete with XX/first-weight tiles for head HBM bandwidth.
            w1t = {}

            def load_w1():
                for hc in range(NHC):
                    wt1 = xpool.tile([128, DIM], bf, tag=f"w1_{hc}", name=f"w1_{hc}")
                    nc.gpsimd.dma_start(wt1[:], w1_d[hc * 128 : (hc + 1) * 128, :])
                    w1t[hc] = wt1

            def load_wpair(k):
                """One contiguous 2-pair weight DMA per w (scalar / gpsimd)."""
                wtiles = []
                for w in (0, 1):
                    wt = wts.tile(
                        [128, 2, 2, HLOC], f8, tag=f"wf8_{w}", name=f"wf{w}_{k}"
                    )
                    eng = nc.gpsimd if w == 1 else nc.scalar
                    eng.dma_start(
                        wt[:],
                        wf8_d[w, 2 * k : 2 * k + 2].rearrange("k p j h -> p k j h"),
                    )
                    wtiles.append(wt)
                return wtiles

            def consume(q, pt3, wtiles, idx, ws=(0, 1)):
                """DR matmuls accumulating pair q from wtiles[w][:, idx]."""
                st = q == 0
                sp = q == NPAIR - 1
                # hc-major on the last pair so the per-hc epilogue chains can
                # start as soon as their accumulators stop.
                order = (
                    [(w, hc) for hc in range(NHC) for w in ws]
                    if sp
                    else [(w, hc) for w in ws for hc in range(NHC)]
                )
                for w, hc in order:
                    hsl = slice(hc * 128, (hc + 1) * 128)
                    nc.tensor.matmul(
                        acc[(w, hc)][:],
                        wtiles[w][:, idx, :, hsl],
                        pt3[:],
                        start=st,
                        stop=sp,
                        perf_mode=DR,
                    )

            # kpairs 0+1 products: q1 squares, then XX2 (DVE-built), q2, q66.
            # Their matmuls interleave w2-before-w3 across both kpairs so the
            # PE has ~3.4us of w2 work while the first gpsimd (w3) weight
            # transfers land.
            wts_k0 = load_wpair(0)
            wts_k1 = load_wpair(1)
            pt1 = poly.tile([128, 2 * TLOC], f8, tag="poly", name="pt1")
            nc.vector.tensor_mul(pt1[:], XX[:], XX[:])
            pt2 = poly.tile([128, 2 * TLOC], f8, tag="poly", name="pt2")
            nc.vector.tensor_mul(pt2[:, 0:TLOC], XX[:, 0:TLOC], XX[:, TLOC : 2 * TLOC])
            nc.gpsimd.memset(pt2[:, TLOC : 2 * TLOC], 0.0)
            sw64 = shift.tile([128, 2 * TLOC], f8, tag="sd", name="sw64")
            nc.sync.dma_start(
                sw64.rearrange("p (j t) -> p j t", j=2),
                xt_d[64 : 64 + 256, :].rearrange("(j p) t -> p j t", p=128),
            )
            pt66 = poly.tile([128, 2 * TLOC], f8, tag="poly", name="pt66")
            nc.vector.tensor_mul(pt66[:], XX[:], sw64[:])
            pt1v = pt1.rearrange("p (j t) -> p j t", j=2)
            pt2v = pt2.rearrange("p (j t) -> p j t", j=2)
            pt66v = pt66.rearrange("p (j t) -> p j t", j=2)
            consume(0, XXv, wts_k0, 0, ws=(0,))
            consume(1, pt1v, wts_k0, 1, ws=(0,))
            consume(0, XXv, wts_k0, 0, ws=(1,))
            consume(1, pt1v, wts_k0, 1, ws=(1,))
            consume(2, pt2v, wts_k1, 0, ws=(0,))
            consume(66, pt66v, wts_k1, 1, ws=(0,))
            consume(2, pt2v, wts_k1, 0, ws=(1,))
            consume(66, pt66v, wts_k1, 1, ws=(1,))

            # superbatches (d, d+64), d = 1..63
            for d in range(1, 64):
                k = d + 1
                if k == 12:
                    load_w1()
                wts_k = load_wpair(k)
                sw = shift.tile([128, 4 * TLOC], f8, tag="sd", name=f"sw{d}")
                nc.sync.dma_start(
                    sw.rearrange("p (m t) -> p m t", m=4), shift_src(d)
                )
                ps = poly.tile([128, 4 * TLOC], f8, tag="poly", name=f"ps{d}")
                psv = ps.rearrange("p (j s t) -> p j s t", j=2, s=2)
                if d == 1:
                    # first superbatch: two s-half multiplies straight off XX,
                    # so ps1 doesn't wait for the XX2 build. The XX2 copies
                    # (DVE; gpsimd's COPY is slow and steals DVE SBUF ports)
                    # follow — they're only needed from ps2 on.
                    swv = sw.rearrange("p (j s t) -> p j s t", j=2, s=2)
                    for s in (0, 1):
                        nc.vector.tensor_mul(psv[:, :, s], XXv[:], swv[:, :, s])
                    for s in (0, 1):
                        nc.vector.tensor_copy(XX2v[:, :, s], XXv[:])
                else:
                    nc.vector.tensor_mul(ps[:], XX2[:], sw[:])
                consume(d + 2, psv[:, :, 0], wts_k, 0)
                consume(d + 66, psv[:, :, 1], wts_k, 1)

            # epilogue per h-chunk: gated = silu(gate/256) * value_raw, bf16.
            # value's 1/256 is folded into w1 host-side, so the DVE multiply
            # reads acc1 (PSUM) directly — no tensor_scalar pass.
            gated = {}
            for hc in range(NHC):
                sil = epi.tile([128, TLOC], bf, tag=f"sil{hc}", name=f"sil{hc}")
                g = epi.tile([128, TLOC], bf, tag=f"gated{hc}", name=f"g{hc}")
                nc.scalar.activation(
                    sil[:], acc[(0, hc)][:], AF.Silu, scale=1.0 / W_SCALE
                )
                nc.vector.tensor_mul(g[:], sil[:], acc[(1, hc)][:])
                gated[hc] = g

            # out matmuls hc-outer: wave hc starts as soon as gated[hc] exists
            NTC = TLOC // 128
            ops = {}
            for tc_i in range(NTC):
                ops[tc_i] = psum.tile(
                    [128, DIM],
                    f32,
                    tag=f"acc{tc_i % 2}{(tc_i // 2) % 2}",
                    name=f"ops{tc_i}",
                )
            for hc in range(NHC):
                for tc_i in range(NTC):
                    tsl = slice(tc_i * 128, (tc_i + 1) * 128)
                    nc.tensor.matmul(
                        ops[tc_i][:],
                        gated[hc][:, tsl],
                        w1t[hc][:],
                        start=hc == 0,
                        stop=hc == NHC - 1,
                    )
            # stage + store: alternate ACT/DVE for the PSUM reads. Output DMAs
            # avoid the gpsimd queue so its slow end-of-kernel drain starts
            # right after the last w3 tile and hides under the tail.
            oq = [nc.sync, nc.scalar, nc.sync, nc.scalar]
            for tc_i in range(NTC):
                tsl = slice(tc_i * 128, (tc_i + 1) * 128)
                ost = ostage.tile([128, DIM], f32, tag="ost", name=f"ost{tc_i}")
                if tc_i % 2 == 0:
                    nc.scalar.copy(ost[:], ops[tc_i][:])
                else:
                    nc.vector.tensor_copy(ost[:], ops[tc_i][:])
                oq[tc_i % 4].dma_start(out_d[tsl, :], ost[:])

    nc.finalize()
    return nc


def _get_nc():
    global _NC_CACHE
    if _NC_CACHE is None:
        _NC_CACHE = _build_nc()
    return _NC_CACHE


def prepare_inputs(x, w1, w2, w3):
    """Host-side shard prep. Returns in_maps for the 8 cores."""
    perm = build_perm()  # (260*128,) with -1 for pad rows
    xt1 = np.ascontiguousarray(x.reshape(T, DIM).T).astype(FP8)  # (256, 2048)
    xt2 = np.concatenate([xt1, xt1], axis=0)  # (512, 2048)
    worder = np.asarray(WORDER)

    def to_pairs(w):  # (HIDDEN, POLY) -> (NPAIR, 128, 2, HIDDEN) f32 scaled
        wt = w.T * W_SCALE  # (POLY, HIDDEN)
        wt = np.concatenate([wt, np.zeros((1, HIDDEN), wt.dtype)], axis=0)
        g = wt[perm]  # perm -1 -> last (zero) row
        # row layout: pair q, tile j, partition k  ->  row (2q+j)*128+k
        pairs = g.reshape(NPAIR, 2, 128, HIDDEN).transpose(0, 2, 1, 3)
        return pairs[worder]  # device processing order

    w2p = to_pairs(w2)
    w3p = to_pairs(w3)
    # value path's 1/W_SCALE descale is folded into w1 (the epilogue multiplies
    # silu(gate) by the raw, W_SCALE-scaled value accumulator)
    w1t = np.ascontiguousarray(w1.T / W_SCALE).astype(BF16)  # (1024, 256)

    in_maps = []
    for c in range(NCORES):
        tg, hg = divmod(c, NHG)
        tsl = slice(tg * TLOC, (tg + 1) * TLOC)
        hsl = slice(hg * HLOC, (hg + 1) * HLOC)
        wf8 = np.stack([w2p[:, :, :, hsl], w3p[:, :, :, hsl]]).astype(FP8)
        in_maps.append(
            {
                "xt": np.ascontiguousarray(xt2[:, tsl]),
                "wf8": np.ascontiguousarray(wf8),
                "w1s": np.ascontiguousarray(w1t[hsl, :]),
            }
        )
    return in_maps


def run(x, w1, w2, w3, trace=False, trace_kwargs=None):
    from concourse.bass_utils import run_bass_kernel_spmd

    nc = _get_nc()
    in_maps = prepare_inputs(x, w1, w2, w3)
    last_err = None
    for attempt in range(3):
        try:
            res = run_bass_kernel_spmd(
                nc,
                in_maps,
                core_ids=list(range(NCORES)),
                trace=trace,
                **(trace_kwargs or {}),
            )
            break
        except Exception as e:  # transient device wedge (e.g. NRT unrecoverable)
            last_err = e
            import time as _time

            _time.sleep(5)
    else:
        raise last_err
    out = np.empty((T, DIM), dtype=np.float64)
    for tg in range(NTG):
        tsl = slice(tg * TLOC, (tg + 1) * TLOC)
        accs = np.zeros((TLOC, DIM), dtype=np.float64)
        for hg in range(NHG):
            accs += res.results[tg * NHG + hg]["out"].astype(np.float64)
        out[tsl] = x.reshape(T, DIM)[tsl].astype(np.float64) + accs
    return out.astype(np.float32).reshape(x.shape), res


def kernel(x, w1, w2, w3):
    out, _ = run(np.asarray(x), np.asarray(w1), np.asarray(w2), np.asarray(w3))
    return out


# revision 33
# speedup vs baseline: 1.0226x; 1.0073x over previous
"""Trainium2 Bass kernel for AtlasMemoryPoly (dense_mlp).

Reference (DIM=256, HIDDEN=1024, POLY=33152, x:(2,1024,256)):
    x_poly = [x, x_i*x_j for i<=j]                  # (T=2048, P=33152)
    gate   = silu(x_poly @ w2.T)                    # (T, H)
    value  = x_poly @ w3.T                          # (T, H)
    out    = x + (gate*value) @ w1.T                # (T, D)

Sharding: 8 cores = 4 t-groups (512 each) x 2 h-groups (512 each).
Each core computes its (t_local, h_local) block and a partial output
(512, 256); the host sums the 2 h-group partials per t-group, adds x.

Poly features: the 33152 poly axis is PERMUTED host-side (same
permutation applied to w2/w3 rows) into PAIRS of 128-feature tiles.
Pair q covers tiles (2q, 2q+1), generated as elementwise multiplies
of stacked row-window tiles (partition p, j in {0,1}):
    q0: [X0;X1] copy              (linear features)
    q1: [X0;X1] * [X0;X1]         (squares)
    q2: [X0*X1 ; ZERO-PAD]        (antipodal + pad to 260 tiles)
    q(2+d), d=1..127: [X0;X1] * xt2[d:256+d]
          j=0: X0*xT[d:d+128]      -> pairs (i, i+d)
          j=1: X1*xT[128+d:256+d]  -> pairs (128+i, (128+i+d)%256)
xt2 is xT doubled (512 rows) so every window is a strided DMA gather.

Processing order (worder): [q0,q1,q2,q66] then superbatches (d, d+64)
for d=1..63 — each superbatch is ONE 4-D strided DMA (overlapping
windows d and d+64 expressed via a hand-built access pattern), ONE DVE
multiply against XX2=[XX|XX], and ONE contiguous 2-pair weight DMA per
w (host packs w2/w3 pair-blocks in worder). This halves DMA instruction
counts and doubles HWDGE in-flight slack vs per-pair transfers.

All products are written fp8e4 and consumed by DoubleRow matmuls
(K=256, 2 MACs/PE/cycle — the fp8 peak). w2/w3 are host-scaled by 256
(fp8e4 range); the gate path descales via ACT silu scale, the value
path's 1/256 is folded into w1.

Head: XX is loaded as 4 chunks, one leading each DMA queue, so the
weight-prefetch flood can't starve it; XX2 is built SBUF->SBUF. A few
dummy warm-up matmuls run during the DMA head to engage the PE HAM
clock-gate early. Tail: hc-outer output matmuls overlap the epilogue,
PSUM drains alternate ACT/DVE, output DMAs avoid the gpsimd queue so
its slow end-of-kernel drain hides under the tail.
"""

import sys

sys.path.insert(0, "/opt/trn_rl_repo")

import numpy as np
import ml_dtypes

DIM = 256
HIDDEN = 1024
T = 2048
POLY = DIM + DIM * (DIM + 1) // 2  # 33152
NPAIR = 130  # 260 tiles of 128 (one zero pad tile)
NCORES = 8
NHG = 2
NTG = 4
HLOC = HIDDEN // NHG  # 512
TLOC = T // NTG  # 512
NHC = HLOC // 128  # 4 h-chunks
W_SCALE = 256.0
WARMUP_MM = 7  # dummy matmuls during the DMA head to pre-warm the PE clock gate
# Ring depths bound the HEAD's HBM flood (every free slot issues its DMA at
# t=0, competing with the critical XX/shift chain) while steady-state lead
# stays ample: one kpair is consumed per ~3.4us, so ring 4 = ~13.8us of lead
# vs ~1.3us transfer time.
WTS_BUFS = 4  # weight kpair-tile prefetch ring depth (2KB/partition each)
SHIFT_BUFS = 5  # shift superbatch-window ring depth
POLY_BUFS = 8  # poly product-tile ring depth

# pair processing order: specials, then superbatch pairs (d, d+64), d=1..63
WORDER = [0, 1, 2, 66] + [q for d in range(1, 64) for q in (d + 2, d + 66)]

BF16 = ml_dtypes.bfloat16
FP8 = ml_dtypes.float8_e4m3fn


def build_perm():
    """tile-row index (260*128) -> old poly row, or -1 for the pad tile.

    Tile order: [lin0, lin1, sq0, sq1, anti, PAD, then (A_d, B_d) for
    d=1..127] where A_d rows i are pairs (i, i+d) and B_d rows i are
    pairs (128+i, (128+i+d) % 256).
    """
    i = np.arange(128)

    def pairs_to_old(a, b):
        lo = np.minimum(a, b)
        hi = np.maximum(a, b)
        return DIM + lo * DIM - lo * (lo - 1) // 2 + (hi - lo)

    chunks = [
        np.arange(0, 128),                  # lin0
        np.arange(128, 256),                # lin1
        pairs_to_old(i, i),                 # sq0
        pairs_to_old(128 + i, 128 + i),     # sq1
        pairs_to_old(i, 128 + i),           # anti
        np.full(128, -1, dtype=np.int64),   # PAD
    ]
    for d in range(1, 128):
        chunks.append(pairs_to_old(i, i + d))
        j = (128 + i + d) % 256
        chunks.append(pairs_to_old(128 + i, j))
    return np.concatenate(chunks)


_NC_CACHE = None


def _build_nc():
    from concourse import bacc, tile, mybir
    from concourse.mybir import ActivationFunctionType as AF
    from bass_rust import VecI64Pair

    nc = bacc.Bacc()
    bf = mybir.dt.bfloat16
    f8 = mybir.dt.float8e4
    f32 = mybir.dt.float32
    DR = mybir.MatmulPerfMode.DoubleRow

    xt_d = nc.dram_tensor("xt", (2 * DIM, TLOC), f8, kind="ExternalInput")
    wf8_d = nc.dram_tensor("wf8", (2, NPAIR, 128, 2, HLOC), f8, kind="ExternalInput")
    # w1 as fp8 DoubleRow pairs: (hc-pair, partition, ko, d)
    w1_d = nc.dram_tensor("w1s", (2, 128, 2, DIM), f8, kind="ExternalInput")
    out_d = nc.dram_tensor("out", (TLOC, DIM), f32, kind="ExternalOutput")

    def shift_src(d):
        """3-D DRAM gather: rows d + p + 64*m (m=0..3), cols t.

        With tile layout (p, j, s, t) — j outer, s inner — m = 2j+s, so
        windows d (s=0) and d+64 (s=1) interleave affinely: m=0 -> window
        d half j0, m=1 -> window d+64 half j0, m=2 -> d half j1, m=3 ->
        d+64 half j1. Overlapping windows in one DMA via a hand-built AP.
        """
        ap = xt_d[d : d + 128, :].unsqueeze(1).copy()
        ap.ap = VecI64Pair([[TLOC, 128], [64 * TLOC, 4], [1, TLOC]])
        return ap

    with tile.TileContext(nc) as tc:
        with (
            tc.tile_pool(name="xpool", bufs=1) as xpool,
            tc.tile_pool(name="shift", bufs=SHIFT_BUFS) as shift,
            tc.tile_pool(name="poly", bufs=POLY_BUFS) as poly,
            tc.tile_pool(name="wts", bufs=WTS_BUFS) as wts,
            tc.tile_pool(name="epi", bufs=1) as epi,
            tc.tile_pool(name="ostage", bufs=4) as ostage,
            tc.tile_pool(name="psum", bufs=1, space="PSUM") as psum,
        ):
            # XX = [X0; X1] stacked pair tile (128, 2*TLOC): j-major halves.
            # One DMA leading the sync queue (everything depends on it; the
            # weight queues' floods are bounded by their ring depths).
            XX = xpool.tile([128, 2 * TLOC], f8, tag="XX")
            XXv = XX.rearrange("p (j t) -> p j t", j=2)
            nc.sync.dma_start(
                XXv[:], xt_d[0:256, :].rearrange("(j p) t -> p j t", p=128)
            )
            # XX2: j-outer/s-inner interleave [X0|X0|X1|X1] matching the
            # superbatch window layout. Built by the DVE (keeps both the HBM
            # and the DMA queues out of the critical head path); emitted
            # after pt1 below so q1's product comes first.
            XX2 = xpool.tile([128, 4 * TLOC], f8, tag="XX2")
            XX2v = XX2.rearrange("p (j s t) -> p j s t", j=2, s=2)

            acc = {}
            for w in (0, 1):
                for hc in range(NHC):
                    acc[(w, hc)] = psum.tile(
                        [128, TLOC], f32, tag=f"acc{w}{hc}", name=f"acc{w}{hc}"
                    )

            if WARMUP_MM:
                # PE HAM clock-gate warmup: dummy matmuls on a zeroed tile
                # while the first DMAs are in flight. q0's start=True
                # re-clears the accumulator, so results are discarded.
                wu = xpool.tile([128, 512], bf, tag="warmup")
                nc.gpsimd.memset(wu[:], 0.0)
                for i in range(WARMUP_MM):
                    nc.tensor.matmul(
                        acc[(0, 0)][:],
                        wu[:, 0:128],
                        wu[:],
                        start=True,
                        stop=True,
                        skip_group_check=True,
                    )

            # w1 tiles are loaded mid-stream (see kpair loop) so their DMAs
            # don't compete with XX/first-weight tiles for head HBM bandwidth.
            w1t = {}

            def load_w1():
                for hcp in (0, 1):
                    wt1 = xpool.tile(
                        [128, 2, DIM], f8, tag=f"w1_{hcp}", name=f"w1_{hcp}"
                    )
                    nc.gpsimd.dma_start(wt1[:], w1_d[hcp])
                    w1t[hcp] = wt1

            def load_wpair(k):
                """One contiguous 2-pair weight DMA per w (scalar / gpsimd)."""
                wtiles = []
                for w in (0, 1):
                    wt = wts.tile(
                        [128, 2, 2, HLOC], f8, tag=f"wf8_{w}", name=f"wf{w}_{k}"
                    )
                    eng = nc.gpsimd if w == 1 else nc.scalar
                    eng.dma_start(
                        wt[:],
                        wf8_d[w, 2 * k : 2 * k + 2].rearrange("k p j h -> p k j h"),
                    )
                    wtiles.append(wt)
                return wtiles

            def consume(q, pt3, wtiles, idx, ws=(0, 1)):
                """DR matmuls accumulating pair q from wtiles[w][:, idx]."""
                st = q == 0
                sp = q == NPAIR - 1
                # hc-major on the last pair so the per-hc epilogue chains can
                # start as soon as their accumulators stop.
                order = (
                    [(w, hc) for hc in range(NHC) for w in ws]
                    if sp
                    else [(w, hc) for w in ws for hc in range(NHC)]
                )
                for w, hc in order:
                    hsl = slice(hc * 128, (hc + 1) * 128)
                    nc.tensor.matmul(
                        acc[(w, hc)][:],
                        wtiles[w][:, idx, :, hsl],
                        pt3[:],
                        start=st,
                        stop=sp,
                        perf_mode=DR,
                    )

            # kpairs 0+1 products: q1 squares, then XX2 (DVE-built), q2, q66.
            # Their matmuls interleave w2-before-w3 across both kpairs so the
            # PE has ~3.4us of w2 work while the first gpsimd (w3) weight
            # transfers land.
            wts_k0 = load_wpair(0)
            wts_k1 = load_wpair(1)
            pt1 = poly.tile([128, 2 * TLOC], f8, tag="poly", name="pt1")
            nc.vector.tensor_mul(pt1[:], XX[:], XX[:])
            pt2 = poly.tile([128, 2 * TLOC], f8, tag="poly", name="pt2")
            nc.vector.tensor_mul(pt2[:, 0:TLOC], XX[:, 0:TLOC], XX[:, TLOC : 2 * TLOC])
            nc.gpsimd.memset(pt2[:, TLOC : 2 * TLOC], 0.0)
            sw64 = shift.tile([128, 2 * TLOC], f8, tag="sd", name="sw64")
            nc.sync.dma_start(
                sw64.rearrange("p (j t) -> p j t", j=2),
                xt_d[64 : 64 + 256, :].rearrange("(j p) t -> p j t", p=128),
            )
            pt66 = poly.tile([128, 2 * TLOC], f8, tag="poly", name="pt66")
            nc.vector.tensor_mul(pt66[:], XX[:], sw64[:])
            pt1v = pt1.rearrange("p (j t) -> p j t", j=2)
            pt2v = pt2.rearrange("p (j t) -> p j t", j=2)
            pt66v = pt66.rearrange("p (j t) -> p j t", j=2)
            consume(0, XXv, wts_k0, 0, ws=(0,))
            consume(1, pt1v, wts_k0, 1, ws=(0,))
            consume(0, XXv, wts_k0, 0, ws=(1,))
            consume(1, pt1v, wts_k0, 1, ws=(1,))
            consume(2, pt2v, wts_k1, 0, ws=(0,))
            consume(66, pt66v, wts_k1, 1, ws=(0,))
            consume(2, pt2v, wts_k1, 0, ws=(1,))
            consume(66, pt66v, wts_k1, 1, ws=(1,))

            # superbatches (d, d+64), d = 1..63
            for d in range(1, 64):
                k = d + 1
                if k == 12:
                    load_w1()
                wts_k = load_wpair(k)
                sw = shift.tile([128, 4 * TLOC], f8, tag="sd", name=f"sw{d}")
                nc.sync.dma_start(
                    sw.rearrange("p (m t) -> p m t", m=4), shift_src(d)
                )
                ps = poly.tile([128, 4 * TLOC], f8, tag="poly", name=f"ps{d}")
                psv = ps.rearrange("p (j s t) -> p j s t", j=2, s=2)
                if d == 1:
                    # first superbatch: two s-half multiplies straight off XX,
                    # so ps1 doesn't wait for the XX2 build. The XX2 copies
                    # (DVE; gpsimd's COPY is slow and steals DVE SBUF ports)
                    # follow — they're only needed from ps2 on.
                    swv = sw.rearrange("p (j s t) -> p j s t", j=2, s=2)
                    for s in (0, 1):
                        nc.vector.tensor_mul(psv[:, :, s], XXv[:], swv[:, :, s])
                    for s in (0, 1):
                        nc.vector.tensor_copy(XX2v[:, :, s], XXv[:])
                else:
                    nc.vector.tensor_mul(ps[:], XX2[:], sw[:])
                consume(d + 2, psv[:, :, 0], wts_k, 0)
                consume(d + 66, psv[:, :, 1], wts_k, 1)

            # epilogue per h-chunk: g2 = (value_raw/16) * silu(gate/256) in
            # fp8, packed as DoubleRow (p, ko, t) pairs of h-chunks. The
            # scale bookkeeping: g2 = 16*gated_true, w1 host-scaled by 8192,
            # so the out matmul result is 131072x the true mlp output — the
            # PSUM drain divides it back out.
            g2 = {}
            for hcp in (0, 1):
                g2[hcp] = epi.tile(
                    [128, 2, TLOC], f8, tag=f"g2_{hcp}", name=f"g2_{hcp}"
                )
            for hc in range(NHC):
                sil = epi.tile([128, TLOC], bf, tag=f"sil{hc}", name=f"sil{hc}")
                nc.scalar.activation(
                    sil[:], acc[(0, hc)][:], AF.Silu, scale=1.0 / W_SCALE
                )
                nc.vector.scalar_tensor_tensor(
                    g2[hc // 2][:, hc % 2, :],
                    acc[(1, hc)][:],
                    1.0 / 16.0,
                    sil[:],
                    mybir.AluOpType.mult,
                    mybir.AluOpType.mult,
                )

            # out matmuls hcp-outer DR fp8: wave hcp starts once g2[hcp] exists
            NTC = TLOC // 128
            OF = 131072.0
            ops = {}
            for tc_i in range(NTC):
                ops[tc_i] = psum.tile(
                    [128, DIM],
                    f32,
                    tag=f"acc{tc_i % 2}{(tc_i // 2) % 2}",
                    name=f"ops{tc_i}",
                )
            for hcp in (0, 1):
                for tc_i in range(NTC):
                    tsl = slice(tc_i * 128, (tc_i + 1) * 128)
                    nc.tensor.matmul(
                        ops[tc_i][:],
                        g2[hcp][:, :, tsl],
                        w1t[hcp][:],
                        start=hcp == 0,
                        stop=hcp == 1,
                        perf_mode=DR,
                    )
            # stage + store: alternate ACT/DVE for the scaled PSUM drains.
            # Output DMAs avoid the gpsimd queue so its slow end-of-kernel
            # drain starts right after the last w3 tile and hides in the tail.
            oq = [nc.sync, nc.scalar, nc.sync, nc.scalar]
            for tc_i in range(NTC):
                tsl = slice(tc_i * 128, (tc_i + 1) * 128)
                ost = ostage.tile([128, DIM], f32, tag="ost", name=f"ost{tc_i}")
                if tc_i % 2 == 0:
                    nc.scalar.activation(
                        ost[:], ops[tc_i][:], AF.Copy, scale=1.0 / OF
                    )
                else:
                    nc.vector.tensor_scalar_mul(ost[:], ops[tc_i][:], 1.0 / OF)
                oq[tc_i % 4].dma_start(out_d[tsl, :], ost[:])

    nc.finalize()
    return nc


def _get_nc():
    global _NC_CACHE
    if _NC_CACHE is None:
        _NC_CACHE = _build_nc()
    return _NC_CACHE


def prepare_inputs(x, w1, w2, w3):
    """Host-side shard prep. Returns in_maps for the 8 cores."""
    perm = build_perm()  # (260*128,) with -1 for pad rows
    xt1 = np.ascontiguousarray(x.reshape(T, DIM).T).astype(FP8)  # (256, 2048)
    xt2 = np.concatenate([xt1, xt1], axis=0)  # (512, 2048)
    worder = np.asarray(WORDER)

    def to_pairs(w):  # (HIDDEN, POLY) -> (NPAIR, 128, 2, HIDDEN) f32 scaled
        wt = w.T * W_SCALE  # (POLY, HIDDEN)
        wt = np.concatenate([wt, np.zeros((1, HIDDEN), wt.dtype)], axis=0)
        g = wt[perm]  # perm -1 -> last (zero) row
        # row layout: pair q, tile j, partition k  ->  row (2q+j)*128+k
        pairs = g.reshape(NPAIR, 2, 128, HIDDEN).transpose(0, 2, 1, 3)
        return pairs[worder]  # device processing order

    w2p = to_pairs(w2)
    w3p = to_pairs(w3)
    # w1 in fp8 at x8192 (values land mid-range of e4m3); together with the
    # g2 = 16*gated_true scale, the device divides by 131072 at PSUM drain
    w1f = w1.T * 8192.0  # (1024, 256)

    in_maps = []
    for c in range(NCORES):
        tg, hg = divmod(c, NHG)
        tsl = slice(tg * TLOC, (tg + 1) * TLOC)
        hsl = slice(hg * HLOC, (hg + 1) * HLOC)
        wf8 = np.stack([w2p[:, :, :, hsl], w3p[:, :, :, hsl]]).astype(FP8)
        w1s = w1f[hsl].reshape(2, 2, 128, DIM).transpose(0, 2, 1, 3)
        in_maps.append(
            {
                "xt": np.ascontiguousarray(xt2[:, tsl]),
                "wf8": np.ascontiguousarray(wf8),
                "w1s": np.ascontiguousarray(w1s).astype(FP8),
            }
        )
    return in_maps


def run(x, w1, w2, w3, trace=False, trace_kwargs=None):
    from concourse.bass_utils import run_bass_kernel_spmd

    nc = _get_nc()
    in_maps = prepare_inputs(x, w1, w2, w3)
    last_err = None
    for attempt in range(3):
        try:
            res = run_bass_kernel_spmd(
                nc,
                in_maps,
                core_ids=list(range(NCORES)),
                trace=trace,
                **(trace_kwargs or {}),
            )
            break
        except Exception as e:  # transient device wedge (e.g. NRT unrecoverable)
            last_err = e
            import time as _time

            _time.sleep(5)
    else:
        raise last_err
    out = np.empty((T, DIM), dtype=np.float64)
    for tg in range(NTG):
        tsl = slice(tg * TLOC, (tg + 1) * TLOC)
        accs = np.zeros((TLOC, DIM), dtype=np.float64)
        for hg in range(NHG):
            accs += res.results[tg * NHG + hg]["out"].astype(np.float64)
        out[tsl] = x.reshape(T, DIM)[tsl].astype(np.float64) + accs
    return out.astype(np.float32).reshape(x.shape), res


def kernel(x, w1, w2, w3):
    out, _ = run(np.asarray(x), np.asarray(w1), np.asarray(w2), np.asarray(w3))
    return out


# revision 34
# speedup vs baseline: 1.2087x; 1.1820x over previous
"""Trainium2 Bass kernel for AtlasMemoryPoly (dense_mlp).

Reference (DIM=256, HIDDEN=1024, POLY=33152, x:(2,1024,256)):
    x_poly = [x, x_i*x_j for i<=j]                  # (T=2048, P=33152)
    gate   = silu(x_poly @ w2.T)                    # (T, H)
    value  = x_poly @ w3.T                          # (T, H)
    out    = x + (gate*value) @ w1.T                # (T, D)

Sharding: 8 cores = 4 t-groups (512 each) x 2 h-groups (512 each).
Each core computes its (t_local, h_local) block and a partial output
(512, 256); the host sums the 2 h-group partials per t-group, adds x.

Poly features: the 33152 poly axis is PERMUTED host-side (same
permutation applied to w2/w3 rows) into PAIRS of 128-feature tiles.
Pair q covers tiles (2q, 2q+1), generated as elementwise multiplies
of stacked row-window tiles (partition p, j in {0,1}):
    q0: [X0;X1] copy              (linear features)
    q1: [X0;X1] * [X0;X1]         (squares)
    q2: [X0*X1 ; ZERO-PAD]        (antipodal + pad to 260 tiles)
    q(2+d), d=1..127: [X0;X1] * xt2[d:256+d]
          j=0: X0*xT[d:d+128]      -> pairs (i, i+d)
          j=1: X1*xT[128+d:256+d]  -> pairs (128+i, (128+i+d)%256)
xt2 is xT doubled (512 rows) so every window is a strided DMA gather.

Processing order (worder): [q0,q1,q2,q66] then superbatches (d, d+64)
for d=1..63 — each superbatch is ONE 4-D strided DMA (overlapping
windows d and d+64 expressed via a hand-built access pattern), ONE DVE
multiply against XX2=[XX|XX], and ONE contiguous 2-pair weight DMA per
w (host packs w2/w3 pair-blocks in worder). This halves DMA instruction
counts and doubles HWDGE in-flight slack vs per-pair transfers.

All products are written fp8e4 and consumed by DoubleRow matmuls
(K=256, 2 MACs/PE/cycle — the fp8 peak). w2/w3 are host-scaled by 256
(fp8e4 range); the gate path descales via ACT silu scale. The output
projection is also fp8 DoubleRow: g2 = (value_raw/16)*silu packs
h-chunk pairs, w1 is host-scaled by 8192, and the PSUM drain divides
by 131072.

Head: one XX DMA leads the sync queue; shallow prefetch rings bound
the weight-DMA flood so XX/shift transfers aren't starved; XX2 is
built by the DVE after the first superbatch's direct-from-XX product
(gpsimd COPY is slow and steals DVE SBUF ports — avoid). Dummy
warm-up matmuls bridge the DMA head to engage the PE HAM clock-gate
early and keep it engaged. Tail: hcp-outer output matmuls overlap the
epilogue, scaled PSUM drains alternate ACT/DVE, output DMAs avoid the
gpsimd queue so its slow end-of-kernel drain hides under the tail.
"""

import sys

sys.path.insert(0, "/opt/trn_rl_repo")

import numpy as np
import ml_dtypes

DIM = 256
HIDDEN = 1024
T = 2048
POLY = DIM + DIM * (DIM + 1) // 2  # 33152
NPAIR = 130  # 260 tiles of 128 (one zero pad tile)
NCORES = 8
NHG = 2
NTG = 4
HLOC = HIDDEN // NHG  # 512
TLOC = T // NTG  # 512
NHC = HLOC // 128  # 4 h-chunks
W_SCALE = 256.0
WARMUP_MM = 7  # dummy matmuls during the DMA head to pre-warm the PE clock gate
# Ring depths bound the HEAD's HBM flood (every free slot issues its DMA at
# t=0, competing with the critical XX/shift chain) while steady-state lead
# stays ample: one kpair is consumed per ~3.4us, so ring 4 = ~13.8us of lead
# vs ~1.3us transfer time.
WTS_BUFS = 4  # weight kpair-tile prefetch ring depth (2KB/partition each)
SHIFT_BUFS = 5  # shift superbatch-window ring depth
POLY_BUFS = 8  # poly product-tile ring depth

# pair processing order: specials, then superbatch pairs (d, d+64), d=1..63
WORDER = [0, 1, 2, 66] + [q for d in range(1, 64) for q in (d + 2, d + 66)]

BF16 = ml_dtypes.bfloat16
FP8 = ml_dtypes.float8_e4m3fn


def build_perm():
    """tile-row index (260*128) -> old poly row, or -1 for the pad tile.

    Tile order: [lin0, lin1, sq0, sq1, anti, PAD, then (A_d, B_d) for
    d=1..127] where A_d rows i are pairs (i, i+d) and B_d rows i are
    pairs (128+i, (128+i+d) % 256).
    """
    i = np.arange(128)

    def pairs_to_old(a, b):
        lo = np.minimum(a, b)
        hi = np.maximum(a, b)
        return DIM + lo * DIM - lo * (lo - 1) // 2 + (hi - lo)

    chunks = [
        np.arange(0, 128),                  # lin0
        np.arange(128, 256),                # lin1
        pairs_to_old(i, i),                 # sq0
        pairs_to_old(128 + i, 128 + i),     # sq1
        pairs_to_old(i, 128 + i),           # anti
        np.full(128, -1, dtype=np.int64),   # PAD
    ]
    for d in range(1, 128):
        chunks.append(pairs_to_old(i, i + d))
        j = (128 + i + d) % 256
        chunks.append(pairs_to_old(128 + i, j))
    return np.concatenate(chunks)


_NC_CACHE = None


def _build_nc():
    from concourse import bacc, tile, mybir
    from concourse.mybir import ActivationFunctionType as AF
    from bass_rust import VecI64Pair

    nc = bacc.Bacc()
    bf = mybir.dt.bfloat16
    f8 = mybir.dt.float8e4
    f32 = mybir.dt.float32
    DR = mybir.MatmulPerfMode.DoubleRow

    xt_d = nc.dram_tensor("xt", (2 * DIM, TLOC), f8, kind="ExternalInput")
    wf8_d = nc.dram_tensor("wf8", (2, NPAIR, 128, 2, HLOC), f8, kind="ExternalInput")
    # w1 as fp8 DoubleRow pairs: (hc-pair, partition, ko, d)
    w1_d = nc.dram_tensor("w1s", (2, 128, 2, DIM), f8, kind="ExternalInput")
    out_d = nc.dram_tensor("out", (TLOC, DIM), f32, kind="ExternalOutput")

    def shift_src(d):
        """3-D DRAM gather: rows d + p + 64*m (m=0..3), cols t.

        With tile layout (p, j, s, t) — j outer, s inner — m = 2j+s, so
        windows d (s=0) and d+64 (s=1) interleave affinely: m=0 -> window
        d half j0, m=1 -> window d+64 half j0, m=2 -> d half j1, m=3 ->
        d+64 half j1. Overlapping windows in one DMA via a hand-built AP.
        """
        ap = xt_d[d : d + 128, :].unsqueeze(1).copy()
        ap.ap = VecI64Pair([[TLOC, 128], [64 * TLOC, 4], [1, TLOC]])
        return ap

    with tile.TileContext(nc) as tc:
        with (
            tc.tile_pool(name="xpool", bufs=1) as xpool,
            tc.tile_pool(name="shift", bufs=SHIFT_BUFS) as shift,
            tc.tile_pool(name="poly", bufs=POLY_BUFS) as poly,
            tc.tile_pool(name="wts", bufs=WTS_BUFS) as wts,
            tc.tile_pool(name="epi", bufs=1) as epi,
            tc.tile_pool(name="ostage", bufs=4) as ostage,
            tc.tile_pool(name="psum", bufs=1, space="PSUM") as psum,
        ):
            # XX = [X0; X1] stacked pair tile (128, 2*TLOC): j-major halves.
            # One DMA leading the sync queue (everything depends on it; the
            # weight queues' floods are bounded by their ring depths).
            XX = xpool.tile([128, 2 * TLOC], f8, tag="XX")
            XXv = XX.rearrange("p (j t) -> p j t", j=2)
            nc.sync.dma_start(
                XXv[:], xt_d[0:256, :].rearrange("(j p) t -> p j t", p=128)
            )
            # XX2: j-outer/s-inner interleave [X0|X0|X1|X1] matching the
            # superbatch window layout. Built by the DVE (keeps both the HBM
            # and the DMA queues out of the critical head path); emitted
            # after pt1 below so q1's product comes first.
            XX2 = xpool.tile([128, 4 * TLOC], f8, tag="XX2")
            XX2v = XX2.rearrange("p (j s t) -> p j s t", j=2, s=2)

            acc = {}
            for w in (0, 1):
                for hc in range(NHC):
                    acc[(w, hc)] = psum.tile(
                        [128, TLOC], f32, tag=f"acc{w}{hc}", name=f"acc{w}{hc}"
                    )

            if WARMUP_MM:
                # PE HAM clock-gate warmup: dummy matmuls on a zeroed tile
                # while the first DMAs are in flight. q0's start=True
                # re-clears the accumulator, so results are discarded.
                wu = xpool.tile([128, 512], bf, tag="warmup")
                nc.gpsimd.memset(wu[:], 0.0)
                for i in range(WARMUP_MM):
                    nc.tensor.matmul(
                        acc[(0, 0)][:],
                        wu[:, 0:128],
                        wu[:],
                        start=True,
                        stop=True,
                        skip_group_check=True,
                    )

            # w1 tiles are loaded mid-stream (see kpair loop) so their DMAs
            # don't compete with XX/first-weight tiles for head HBM bandwidth.
            w1t = {}

            def load_w1():
                for hcp in (0, 1):
                    wt1 = xpool.tile(
                        [128, 2, DIM], f8, tag=f"w1_{hcp}", name=f"w1_{hcp}"
                    )
                    nc.gpsimd.dma_start(wt1[:], w1_d[hcp])
                    w1t[hcp] = wt1

            def load_wpair(k):
                """One contiguous 2-pair weight DMA per w (scalar / gpsimd)."""
                wtiles = []
                for w in (0, 1):
                    wt = wts.tile(
                        [128, 2, 2, HLOC], f8, tag=f"wf8_{w}", name=f"wf{w}_{k}"
                    )
                    eng = nc.gpsimd if w == 1 else nc.scalar
                    eng.dma_start(
                        wt[:],
                        wf8_d[w, 2 * k : 2 * k + 2].rearrange("k p j h -> p k j h"),
                    )
                    wtiles.append(wt)
                return wtiles

            def consume(q, pt3, wtiles, idx, ws=(0, 1)):
                """DR matmuls accumulating pair q from wtiles[w][:, idx]."""
                st = q == 0
                sp = q == NPAIR - 1
                # hc-major on the last pair so the per-hc epilogue chains can
                # start as soon as their accumulators stop.
                order = (
                    [(w, hc) for hc in range(NHC) for w in ws]
                    if sp
                    else [(w, hc) for w in ws for hc in range(NHC)]
                )
                for w, hc in order:
                    hsl = slice(hc * 128, (hc + 1) * 128)
                    nc.tensor.matmul(
                        acc[(w, hc)][:],
                        wtiles[w][:, idx, :, hsl],
                        pt3[:],
                        start=st,
                        stop=sp,
                        perf_mode=DR,
                    )

            # kpairs 0+1 products: q1 squares, then XX2 (DVE-built), q2, q66.
            # Their matmuls interleave w2-before-w3 across both kpairs so the
            # PE has ~3.4us of w2 work while the first gpsimd (w3) weight
            # transfers land.
            wts_k0 = load_wpair(0)
            wts_k1 = load_wpair(1)
            pt1 = poly.tile([128, 2 * TLOC], f8, tag="poly", name="pt1")
            nc.vector.tensor_mul(pt1[:], XX[:], XX[:])
            pt2 = poly.tile([128, 2 * TLOC], f8, tag="poly", name="pt2")
            nc.vector.tensor_mul(pt2[:, 0:TLOC], XX[:, 0:TLOC], XX[:, TLOC : 2 * TLOC])
            nc.gpsimd.memset(pt2[:, TLOC : 2 * TLOC], 0.0)
            sw64 = shift.tile([128, 2 * TLOC], f8, tag="sd", name="sw64")
            nc.sync.dma_start(
                sw64.rearrange("p (j t) -> p j t", j=2),
                xt_d[64 : 64 + 256, :].rearrange("(j p) t -> p j t", p=128),
            )
            pt66 = poly.tile([128, 2 * TLOC], f8, tag="poly", name="pt66")
            nc.vector.tensor_mul(pt66[:], XX[:], sw64[:])
            pt1v = pt1.rearrange("p (j t) -> p j t", j=2)
            pt2v = pt2.rearrange("p (j t) -> p j t", j=2)
            pt66v = pt66.rearrange("p (j t) -> p j t", j=2)
            consume(0, XXv, wts_k0, 0, ws=(0,))
            consume(1, pt1v, wts_k0, 1, ws=(0,))
            consume(0, XXv, wts_k0, 0, ws=(1,))
            consume(1, pt1v, wts_k0, 1, ws=(1,))
            consume(2, pt2v, wts_k1, 0, ws=(0,))
            consume(66, pt66v, wts_k1, 1, ws=(0,))
            consume(2, pt2v, wts_k1, 0, ws=(1,))
            consume(66, pt66v, wts_k1, 1, ws=(1,))

            # superbatches (d, d+64), d = 1..63
            for d in range(1, 64):
                k = d + 1
                if k == 12:
                    load_w1()
                wts_k = load_wpair(k)
                sw = shift.tile([128, 4 * TLOC], f8, tag="sd", name=f"sw{d}")
                nc.sync.dma_start(
                    sw.rearrange("p (m t) -> p m t", m=4), shift_src(d)
                )
                ps = poly.tile([128, 4 * TLOC], f8, tag="poly", name=f"ps{d}")
                psv = ps.rearrange("p (j s t) -> p j s t", j=2, s=2)
                if d == 1:
                    # first superbatch: two s-half multiplies straight off XX,
                    # so ps1 doesn't wait for the XX2 build. The XX2 copies
                    # (DVE; gpsimd's COPY is slow and steals DVE SBUF ports)
                    # follow — they're only needed from ps2 on.
                    swv = sw.rearrange("p (j s t) -> p j s t", j=2, s=2)
                    for s in (0, 1):
                        nc.vector.tensor_mul(psv[:, :, s], XXv[:], swv[:, :, s])
                    for s in (0, 1):
                        nc.vector.tensor_copy(XX2v[:, :, s], XXv[:])
                else:
                    nc.vector.tensor_mul(ps[:], XX2[:], sw[:])
                consume(d + 2, psv[:, :, 0], wts_k, 0)
                consume(d + 66, psv[:, :, 1], wts_k, 1)

            # epilogue per h-chunk: g2 = (value_raw/16) * silu(gate/256) in
            # fp8, packed as DoubleRow (p, ko, t) pairs of h-chunks. The
            # scale bookkeeping: g2 = 16*gated_true, w1 host-scaled by 8192,
            # so the out matmul result is 131072x the true mlp output — the
            # PSUM drain divides it back out.
            g2 = {}
            for hcp in (0, 1):
                g2[hcp] = epi.tile(
                    [128, 2, TLOC], f8, tag=f"g2_{hcp}", name=f"g2_{hcp}"
                )
            for hc in range(NHC):
                sil = epi.tile([128, TLOC], bf, tag=f"sil{hc}", name=f"sil{hc}")
                nc.scalar.activation(
                    sil[:], acc[(0, hc)][:], AF.Silu, scale=1.0 / W_SCALE
                )
                nc.vector.scalar_tensor_tensor(
                    g2[hc // 2][:, hc % 2, :],
                    acc[(1, hc)][:],
                    1.0 / 16.0,
                    sil[:],
                    mybir.AluOpType.mult,
                    mybir.AluOpType.mult,
                )

            # out matmuls hcp-outer DR fp8: wave hcp starts once g2[hcp] exists
            NTC = TLOC // 128
            OF = 131072.0
            ops = {}
            for tc_i in range(NTC):
                ops[tc_i] = psum.tile(
                    [128, DIM],
                    f32,
                    tag=f"acc{tc_i % 2}{(tc_i // 2) % 2}",
                    name=f"ops{tc_i}",
                )
            for hcp in (0, 1):
                for tc_i in range(NTC):
                    tsl = slice(tc_i * 128, (tc_i + 1) * 128)
                    nc.tensor.matmul(
                        ops[tc_i][:],
                        g2[hcp][:, :, tsl],
                        w1t[hcp][:],
                        start=hcp == 0,
                        stop=hcp == 1,
                        perf_mode=DR,
                    )
            # stage + store: alternate ACT/DVE for the scaled PSUM drains.
            # Output DMAs avoid the gpsimd queue so its slow end-of-kernel
            # drain starts right after the last w3 tile and hides in the tail.
            oq = [nc.sync, nc.scalar, nc.sync, nc.scalar]
            for tc_i in range(NTC):
                tsl = slice(tc_i * 128, (tc_i + 1) * 128)
                ost = ostage.tile([128, DIM], f32, tag="ost", name=f"ost{tc_i}")
                if tc_i % 2 == 0:
                    nc.scalar.activation(
                        ost[:], ops[tc_i][:], AF.Copy, scale=1.0 / OF
                    )
                else:
                    nc.vector.tensor_scalar_mul(ost[:], ops[tc_i][:], 1.0 / OF)
                oq[tc_i % 4].dma_start(out_d[tsl, :], ost[:])

    nc.finalize()
    return nc


def _get_nc():
    global _NC_CACHE
    if _NC_CACHE is None:
        _NC_CACHE = _build_nc()
    return _NC_CACHE


def prepare_inputs(x, w1, w2, w3):
    """Host-side shard prep. Returns in_maps for the 8 cores."""
    perm = build_perm()  # (260*128,) with -1 for pad rows
    xt1 = np.ascontiguousarray(x.reshape(T, DIM).T).astype(FP8)  # (256, 2048)
    xt2 = np.concatenate([xt1, xt1], axis=0)  # (512, 2048)
    worder = np.asarray(WORDER)

    def to_pairs(w):  # (HIDDEN, POLY) -> (NPAIR, 128, 2, HIDDEN) f32 scaled
        wt = w.T * W_SCALE  # (POLY, HIDDEN)
        wt = np.concatenate([wt, np.zeros((1, HIDDEN), wt.dtype)], axis=0)
        g = wt[perm]  # perm -1 -> last (zero) row
        # row layout: pair q, tile j, partition k  ->  row (2q+j)*128+k
        pairs = g.reshape(NPAIR, 2, 128, HIDDEN).transpose(0, 2, 1, 3)
        return pairs[worder]  # device processing order

    w2p = to_pairs(w2)
    w3p = to_pairs(w3)
    # w1 in fp8 at x8192 (values land mid-range of e4m3); together with the
    # g2 = 16*gated_true scale, the device divides by 131072 at PSUM drain
    w1f = w1.T * 8192.0  # (1024, 256)

    in_maps = []
    for c in range(NCORES):
        tg, hg = divmod(c, NHG)
        tsl = slice(tg * TLOC, (tg + 1) * TLOC)
        hsl = slice(hg * HLOC, (hg + 1) * HLOC)
        wf8 = np.stack([w2p[:, :, :, hsl], w3p[:, :, :, hsl]]).astype(FP8)
        w1s = w1f[hsl].reshape(2, 2, 128, DIM).transpose(0, 2, 1, 3)
        in_maps.append(
            {
                "xt": np.ascontiguousarray(xt2[:, tsl]),
                "wf8": np.ascontiguousarray(wf8),
                "w1s": np.ascontiguousarray(w1s).astype(FP8),
            }
        )
    return in_maps


def run(x, w1, w2, w3, trace=False, trace_kwargs=None):
    from concourse.bass_utils import run_bass_kernel_spmd

    nc = _get_nc()
    in_maps = prepare_inputs(x, w1, w2, w3)
    last_err = None
    for attempt in range(3):
        try:
            res = run_bass_kernel_spmd(
                nc,
                in_maps,
                core_ids=list(range(NCORES)),
                trace=trace,
                **(trace_kwargs or {}),
            )
            break
        except Exception as e:  # transient device wedge (e.g. NRT unrecoverable)
            last_err = e
            import time as _time

            _time.sleep(5)
    else:
        raise last_err
    out = np.empty((T, DIM), dtype=np.float64)
    for tg in range(NTG):
        tsl = slice(tg * TLOC, (tg + 1) * TLOC)
        accs = np.zeros((TLOC, DIM), dtype=np.float64)
        for hg in range(NHG):
            accs += res.results[tg * NHG + hg]["out"].astype(np.float64)
        out[tsl] = x.reshape(T, DIM)[tsl].astype(np.float64) + accs
    return out.astype(np.float32).reshape(x.shape), res


def kernel(x, w1, w2, w3):
    out, _ = run(np.asarray(x), np.asarray(w1), np.asarray(w2), np.asarray(w3))
    return out
